# revision 22
# baseline (speedup 1.0000x reference)
"""Causal multi-head attention (B=2, S=2048, D=1024, H=16, HD=64) on 8 trn2 cores.

Sharding: 2 heads per core x both batches (head-parallel QKV/attention/out-proj,
Wo h-split => per-core partial outputs, summed on host).

v2 design notes (vs the 273us baseline):
  - all matmul operands bf16 (PSUM accumulation fp32; softmax denominators,
    reciprocal fp32). Halves SBUF/DMA and shrinks copy costs.
  - host pre-lays-out xt/weights so every DMA is 1 descriptor/partition.
  - score matmuls for the two heads are emitted adjacently at PE row bases
    0/64 (row tiling) so they run concurrently (K=64 each).
  - projections are half-major 8-matmul accumulation chains into 1-bank psum
    tiles from a shared 2-slot ring => PE never waits a full copy latency
    between phases.
  - psum: scores 2x[128,2,512] (4 banks), attnV o 2x[65,512] (2 banks),
    shared "w" ring 2x[128,512] (2 banks) for out-proj/proj-chains/
    broadcast/V-transpose tiles (dedicated ring: pumped V-transposes no
    longer stall on live attention tiles).
  - b1 projections/transposes are pumped into b0's attention stream;
    b0's out-projection (norm-broadcast, scale, 8 matmuls/qj) is DEFERRED
    into b1's attention stream, filling the PE idle left by the exp (ACT)
    dependency. ScalarE is kept exp-only during attention; psum evacuation
    copies go to VectorE (plus ScalarE where it has slack).
"""

import numpy as np
import ml_dtypes

import concourse.bass as bass
import concourse.mybir as mybir
import concourse.tile as tile
from concourse import bacc
from concourse.bass_utils import run_bass_kernel_spmd
from concourse.masks import make_identity
from concourse.dve_ops import RECIPROCAL_APPROX_NR

F32 = mybir.dt.float32
F32R = mybir.dt.float32r
BF16 = mybir.dt.bfloat16
AF = mybir.ActivationFunctionType
NPBF = ml_dtypes.bfloat16

B, S, D, H, HD = 2, 2048, 1024, 16, 64
NCORES = 8
HPC = H // NCORES          # heads per core = 2
HH = HPC * HD              # 128 concat head dims per core
P = 128
DC = D // P                # 8 d-chunks
NQ = 512                   # q tile (psum bank width fp32)
QJ = S // NQ               # 4 q tiles
KC = S // P                # 16 k chunks
GK = 2                     # k-chunks per score group (psum: [128, GK, NQ])
VW = HD + 2                # v row stride (65 used, padded to 66 for alignment)

_NC_CACHE = {}


def _build_nc(with_bias_qkv: bool, with_bias_o: bool, causal: bool):
    key = (with_bias_qkv, with_bias_o, causal)
    if key in _NC_CACHE:
        return _NC_CACHE[key]

    nc = bacc.Bacc("TRN2", target_bir_lowering=False, debug=False)
    xt = nc.dram_tensor("xt", [B, P, DC, S], BF16, kind="ExternalInput")
    wq = nc.dram_tensor("wq", [P, DC, HH], BF16, kind="ExternalInput")
    wk = nc.dram_tensor("wk", [P, DC, HH], BF16, kind="ExternalInput")
    wv = nc.dram_tensor("wv", [P, DC, HH], BF16, kind="ExternalInput")
    wo = nc.dram_tensor("wo", [HH, D], BF16, kind="ExternalInput")
    if with_bias_qkv:
        bqkv = nc.dram_tensor("bqkv", [3, HH], F32, kind="ExternalInput")
    if with_bias_o:
        bo8 = nc.dram_tensor("bo8", [D], F32R, kind="ExternalInput")
    out = nc.dram_tensor("out", [B, S, D], BF16, kind="ExternalOutput")

    with tile.TileContext(nc) as tc:
        with (
            tc.tile_pool(name="const", bufs=1) as cpool,
            tc.tile_pool(name="xtp", bufs=2) as xt_pool,
            tc.tile_pool(name="qkv", bufs=2) as qkv_pool,
            tc.tile_pool(name="otp", bufs=1) as ot_pool,
            tc.tile_pool(name="nrm", bufs=4) as nrm_pool,
            tc.tile_pool(name="ep", bufs=6) as e_pool,
            tc.tile_pool(name="osb", bufs=4) as osb_pool,
            tc.tile_pool(name="ps_s", bufs=2, space="PSUM") as ps_s,
            tc.tile_pool(name="ps_o", bufs=2, space="PSUM") as ps_o,
            tc.tile_pool(name="ps_w", bufs=2, space="PSUM") as ps_w,
        ):
            # ---- constants ----
            # DMA order matters: the first projection packet needs only
            # wq + the first xt chunks, so those go first.
            wq_sb = cpool.tile([P, DC, HH], BF16, tag="wq", name="wq_sb")
            wk_sb = cpool.tile([P, DC, HH], BF16, tag="wk", name="wk_sb")
            wv_sb = cpool.tile([P, DC, HH], BF16, tag="wv", name="wv_sb")
            wo_sb = cpool.tile([P, D], BF16, tag="wo", name="wo_sb")
            nc.sync.dma_start(wq_sb[:], wq[:])
            ones_sb = cpool.tile([33, P], F32R, tag="ones", name="ones_sb")
            nc.vector.memset(ones_sb[:].bitcast(F32), 1.0)

            ident_sb = cpool.tile([P, P], BF16, tag="ident", name="ident_sb")
            make_identity(nc, ident_sb[:])
            # force the exp ACT-table load (~2.7us) during the initial DMA
            # wait instead of ahead of the first real exp
            warm_sb = cpool.tile([1, 8], F32, tag="warm", name="warm_sb")
            nc.scalar.activation(warm_sb[:], ones_sb[0:1, 0:8].bitcast(F32),
                                 AF.Exp, scale=0.125)

            def load_weights_rest():
                nc.sync.dma_start(wk_sb[:], wk[:])
                nc.sync.dma_start(wv_sb[:], wv[:])
                nc.sync.dma_start(wo_sb[:], wo[:])
            if with_bias_qkv:
                # per-partition bias columns: [:, 0]=bq, [:, 1]=bk, [:, 2]=bv
                bqkvt_sb = cpool.tile([HH, 3], F32, tag="bqkvt", name="bqkvt_sb")
                for i in range(3):
                    nc.sync.dma_start(
                        bqkvt_sb[:, i:i + 1], bqkv[i:i + 1, :].rearrange("a f -> f a")
                    )
            if with_bias_o:
                bo8_sb = cpool.tile([1, D], F32R, tag="bo8", name="bo8_sb")
                nc.sync.dma_start(bo8_sb[:], bo8.rearrange("(a d) -> a d", a=1))

            # per-batch state
            st = [dict() for _ in range(B)]

            def load_xt(b):
                xt_sb = xt_pool.tile([P, DC, S], BF16, tag="xt", name="xt_sb")
                for d in range(DC):
                    nc.sync.dma_start(xt_sb[:, d, :], xt[b, :, d, :])
                st[b]["xt"] = xt_sb

            def alloc_qkv(b):
                st[b]["qt"] = qkv_pool.tile([P, QJ, NQ], BF16, tag="qt", name="qt_sb")
                st[b]["kt"] = qkv_pool.tile([P, QJ, NQ], BF16, tag="kt", name="kt_sb")
                st[b]["vt"] = qkv_pool.tile([P, QJ, NQ], BF16, tag="vt", name="vt_sb")
                v_sb = qkv_pool.tile([P, KC, HPC, VW], BF16, tag="v", name="v_sb")
                nc.vector.memset(v_sb[:, :, :, HD:HD + 1], 1.0)
                st[b]["v"] = v_sb

            def proj_copy(dst, j, pp, w_idx, on_act):
                if with_bias_qkv:
                    nc.scalar.activation(
                        dst[:, j, :], pp[:],
                        AF.Identity, bias=bqkvt_sb[:, w_idx:w_idx + 1],
                    )
                elif on_act:
                    nc.scalar.copy(dst[:, j, :], pp[:])
                else:
                    nc.vector.tensor_copy(dst[:, j, :], pp[:])

            def packet(b, w_idx, half, on_act, d_major=False):
                """projection packet: one (w, half) -> two 8-matmul chains.

                d_major: both chains advance together per xt chunk — only for
                the very first packet, which races the xt DMA stream.
                """
                w_sb = (wq_sb, wk_sb, wv_sb)[w_idx]
                dst = (st[b]["qt"], st[b]["kt"], st[b]["vt"])[w_idx]
                if d_major:
                    pps = [ps_w.tile([P, NQ], F32, tag="w", name="proj_ps")
                           for _ in range(2)]
                    for d in range(DC):
                        for j2 in range(2):
                            j = half * 2 + j2
                            nc.tensor.matmul(
                                pps[j2][:],
                                lhsT=w_sb[:, d, :],
                                rhs=st[b]["xt"][:, d, j * NQ:(j + 1) * NQ],
                                start=(d == 0), stop=(d == DC - 1),
                            )
                    for j2 in range(2):
                        proj_copy(dst, half * 2 + j2, pps[j2], w_idx, on_act)
                    return
                for j2 in range(2):
                    j = half * 2 + j2
                    pp = ps_w.tile([P, NQ], F32, tag="w", name="proj_ps")
                    for d in range(DC):
                        nc.tensor.matmul(
                            pp[:],
                            lhsT=w_sb[:, d, :],
                            rhs=st[b]["xt"][:, d, j * NQ:(j + 1) * NQ],
                            start=(d == 0), stop=(d == DC - 1),
                        )
                    proj_copy(dst, j, pp, w_idx, on_act)

            def v_transpose(b, sc):
                tp = ps_w.tile([P, P], BF16, tag="w", name="tr_ps")
                nc.tensor.transpose(
                    tp[:], st[b]["vt"][:, sc // 4, (sc % 4) * P:(sc % 4 + 1) * P],
                    ident_sb[:],
                )
                nc.vector.tensor_copy(
                    st[b]["v"][:, sc, :, :HD],
                    tp.rearrange("p (h d) -> p h d", h=HPC),
                )

            def attn_alloc(b):
                st[b]["ot"] = ot_pool.tile([P, QJ, NQ], BF16, tag=f"ot{b}",
                                           name="ot_sb")
                st[b]["otn"] = ot_pool.tile([P, QJ, NQ], BF16, tag=f"otn{b}",
                                            name="otn_sb")

            def q0_of(qj, ki):
                # causal: chunk ki only reaches q >= ki*P - qj*NQ
                return max(0, ki * P - qj * NQ) if causal else 0

            def score_exp_g(b, qj, g):
                """score matmuls + exp (+ causal mask) for one group.

                Diagonal chunks trim the moving-operand width to the causal
                extent (widths 512/384/256/128) on scores, exp (last group
                only) and attnV; the untouched strips are never read.
                """
                qt, kt = st[b]["qt"], st[b]["kt"]
                stp = [ps_s.tile([P, GK, NQ], F32, tag="st", name=f"st_ps{h}")
                       for h in range(HPC)]
                for c2 in range(GK):
                    ki = g * GK + c2
                    q0 = q0_of(qj, ki)
                    for h in range(HPC):
                        h0 = h * HD
                        nc.tensor.matmul(
                            stp[h][:, c2, q0:],
                            lhsT=kt[h0:h0 + HD, ki // 4, (ki % 4) * P:(ki % 4 + 1) * P],
                            rhs=qt[h0:h0 + HD, qj, q0:],
                            start=True, stop=True,
                        )
                split_exp = causal and q0_of(qj, g * GK) >= NQ // 2
                es = []
                for h in range(HPC):
                    e_sb = e_pool.tile([P, GK, NQ], BF16, tag="e", name="e_sb")
                    if split_exp:
                        for c2 in range(GK):
                            ki = g * GK + c2
                            q0 = q0_of(qj, ki)
                            nc.scalar.activation(
                                e_sb[:, c2, q0:], stp[h][:, c2, q0:],
                                AF.Exp, scale=0.125,
                            )
                            nc.gpsimd.affine_select(
                                out=e_sb[:, c2, q0:], in_=e_sb[:, c2, q0:],
                                compare_op=mybir.AluOpType.is_ge, fill=0.0,
                                base=qj * NQ + q0 - ki * P,
                                pattern=[[1, NQ - q0]],
                                channel_multiplier=-1,
                            )
                    else:
                        nc.scalar.activation(e_sb[:], stp[h][:], AF.Exp, scale=0.125)
                        if causal and g >= 2 * qj:
                            nc.gpsimd.affine_select(
                                out=e_sb[:], in_=e_sb[:],
                                compare_op=mybir.AluOpType.is_ge, fill=0.0,
                                base=qj * NQ - g * GK * P,
                                pattern=[[-P, GK], [1, NQ]],
                                channel_multiplier=-1,
                            )
                    es.append(e_sb)
                return es

            def attnv_g(b, qj, g, ngroups, es, o_ps):
                v = st[b]["v"]
                for h in range(HPC):
                    for c2 in range(GK):
                        ki = g * GK + c2
                        q0 = q0_of(qj, ki)
                        nc.tensor.matmul(
                            o_ps[h][:, q0:],
                            lhsT=v[:, ki, h, :HD + 1],
                            rhs=es[h][:, c2, q0:],
                            start=(g == 0 and c2 == 0),
                            stop=(g == ngroups - 1 and c2 == GK - 1),
                        )

            def attn_qj(b, qj, pump, mid=None):
                """score/exp/attnV for both heads, interleaved group-by-group.

                mid (qj0 only): emitted after the first group's scores/exp but
                before its attnV — lets the V-projection/transposes sit there
                without an engine-order cycle, so the exp stream starts early.
                """
                ngroups = (2 * (qj + 1)) if causal else (KC // GK)
                o_ps = [ps_o.tile([HD + 1, NQ], F32, tag="o", name=f"o_ps{h}")
                        for h in range(HPC)]
                pend = []
                for g in range(ngroups):
                    pend.append((g, score_exp_g(b, qj, g)))
                    if g == 0 and mid is not None:
                        mid()
                    while pend:
                        g2, es = pend.pop(0)
                        attnv_g(b, qj, g2, ngroups, es, o_ps)
                    pump(b, qj, g)
                return o_ps

            def norm_qj(b, qj, o_ps):
                """evacuate o_ps + compute 1/denominator; cheap, on DVE."""
                ot = st[b]["ot"]
                rsq = nrm_pool.tile([33, NQ], F32, tag="rsq", name="rsq")
                for h in range(HPC):
                    nc.vector.tensor_copy(
                        rsq[32 * h:32 * h + 1, :], o_ps[h][HD:HD + 1, :]
                    )
                    nc.vector.tensor_copy(
                        ot[h * HD:(h + 1) * HD, qj, :], o_ps[h][:HD, :]
                    )
                rscq = nrm_pool.tile([33, NQ], F32, tag="rscq", name="rscq")
                rinq = nrm_pool.tile([33, NQ], F32R, tag="rinq", name="rinq")
                nc.vector.reciprocal_approx_fast(out=rscq[:], in_=rsq[:])
                nc.vector._custom_dve(
                    RECIPROCAL_APPROX_NR, out=rinq[:], in0=rsq[:], in1=rscq[:],
                    s0=2.0,
                )
                return rinq

            def outproj_bc(b, qj, rinq):
                """broadcast 1/den across head dims + scale per head."""
                ot, otn = st[b]["ot"], st[b]["otn"]
                bcs = []
                for h in range(HPC):
                    bc = ps_w.tile([HD, NQ], F32, tag="w", name="bc_ps")
                    nc.tensor.matmul(
                        bc[:],
                        lhsT=ones_sb[32 * h:32 * h + 1, :HD],
                        rhs=rinq[32 * h:32 * h + 1, :],
                        start=True, stop=True,
                    )
                    bcs.append(bc)
                for h in range(HPC):
                    h0 = h * HD
                    nc.vector.tensor_mul(
                        otn[h0:h0 + HD, qj, :], ot[h0:h0 + HD, qj, :], bcs[h][:]
                    )

            def outproj_sc(b, qj, sc4, on_act):
                """output projection for one 128-row s-chunk."""
                otn = st[b]["otn"]
                sc = qj * 4 + sc4
                osb = osb_pool.tile([P, 2, NQ], BF16, tag="out", name="out_sb")
                for fc in range(2):
                    op = ps_w.tile([P, NQ], F32, tag="w", name="op_ps")
                    if with_bias_o:
                        nc.tensor.matmul(
                            op[:], lhsT=ones_sb[0:1, :P],
                            rhs=bo8_sb[:, fc * NQ:(fc + 1) * NQ],
                            start=True, stop=False,
                        )
                    nc.tensor.matmul(
                        op[:],
                        lhsT=otn[:, qj, sc4 * P:(sc4 + 1) * P],
                        rhs=wo_sb[:, fc * NQ:(fc + 1) * NQ],
                        start=not with_bias_o, stop=True,
                    )
                    if on_act:
                        nc.scalar.copy(osb[:, fc, :], op[:])
                    else:
                        nc.vector.tensor_copy(osb[:, fc, :], op[:])
                nc.sync.dma_start(
                    out[b, sc * P:(sc + 1) * P, :],
                    osb.rearrange("p a n -> p (a n)"),
                )

            # ---------------- program ----------------
            deferred = []

            def pump_rate(b, qj):
                # b0's attention has inline projection work; b1's is exp-bound
                # with PE slack — drain fast there, leaving the last q-tile
                # just enough to chew through its 8 groups.
                if b == 0:
                    return 1
                return 3 if qj == 0 else 2

            def pump(b, qj, g):
                for _ in range(pump_rate(b, qj)):
                    if deferred:
                        deferred.pop(0)()

            def defer_outproj(b, qj, rinq):
                deferred.append(lambda b=b, q=qj, r=rinq: outproj_bc(b, q, r))
                for sc4 in range(4):
                    deferred.append(
                        lambda b=b, q=qj, s=sc4: outproj_sc(b, q, s, on_act=(s % 2 == 1))
                    )

            load_xt(0)
            load_weights_rest()
            load_xt(1)  # bufs=2: streams in behind b0's chunks
            alloc_qkv(0)
            alloc_qkv(1)
            attn_alloc(0)
            attn_alloc(1)

            def prep_items(b, half, on_act=False):
                its = [lambda w=w, hf=half: packet(b, w, hf, on_act=on_act)
                       for w in range(3)]
                its += [
                    lambda s0=half * 8 + sc2 * 2: (v_transpose(b, s0),
                                                   v_transpose(b, s0 + 1))
                    for sc2 in range(4)
                ]
                return its

            def attn_step(b, qj, mid=None):
                o_ps = attn_qj(b, qj, pump, mid=mid)
                rinq = norm_qj(b, qj, o_ps)
                defer_outproj(b, qj, rinq)

            def b0_v_half0():
                packet(0, 2, 0, on_act=True)
                for sc in range(8):
                    v_transpose(0, sc)

            # b0's attention first (pumping b1's projections/transposes into
            # its stream), then b1's (pumping the deferred out-projections).
            deferred.extend(prep_items(1, 0))
            deferred.extend(prep_items(1, 1))
            packet(0, 0, 0, on_act=True, d_major=True)
            packet(0, 1, 0, on_act=False)
            attn_step(0, 0, mid=b0_v_half0)
            attn_step(0, 1)
            for w_idx in range(3):
                packet(0, w_idx, 1, on_act=(w_idx % 2 == 0))
            for sc in range(8, 16):
                v_transpose(0, sc)
            attn_step(0, 2)
            attn_step(0, 3)
            for qj in reversed(range(QJ)):
                attn_step(1, qj)

            while deferred:
                deferred.pop(0)()

    nc.compile()
    _NC_CACHE[key] = nc
    return nc


def _check_causal(mask: np.ndarray) -> bool:
    m = np.asarray(mask).reshape(mask.shape[-2], mask.shape[-1])
    s = m.shape[0]
    if np.array_equal(m, np.tril(np.ones((s, s), dtype=bool))):
        return True
    if m.all():
        return False
    raise NotImplementedError("only causal or all-true masks are supported")


def kernel(inputs_q, mask, Wq, bq, Wk, bk, Wv, bv, Wo, bo, _trace=False,
           _trace_cores=None):
    inputs_q = np.asarray(inputs_q, dtype=np.float32)
    Wq = np.asarray(Wq, dtype=np.float32).reshape(D, H * HD)
    Wk = np.asarray(Wk, dtype=np.float32).reshape(D, H * HD)
    Wv = np.asarray(Wv, dtype=np.float32).reshape(D, H * HD)
    Wo = np.asarray(Wo, dtype=np.float32).reshape(H * HD, D)
    bq = np.asarray(bq, dtype=np.float32).reshape(H * HD)
    bk = np.asarray(bk, dtype=np.float32).reshape(H * HD)
    bv = np.asarray(bv, dtype=np.float32).reshape(H * HD)
    bo = np.asarray(bo, dtype=np.float32).reshape(D)

    causal = _check_causal(mask)
    with_bias_qkv = bool(bq.any() or bk.any() or bv.any())
    with_bias_o = bool(bo.any())

    nc = _build_nc(with_bias_qkv, with_bias_o, causal)

    # [B, S, D] -> [B, P, DC, S] so each DMA is 1 descriptor per partition
    xt = np.ascontiguousarray(
        inputs_q.transpose(0, 2, 1).reshape(B, DC, P, S).transpose(0, 2, 1, 3)
    ).astype(NPBF)

    def wsplit(w, f0, f1):
        # [D, hh] -> [P, DC, hh]
        return np.ascontiguousarray(
            w[:, f0:f1].reshape(DC, P, f1 - f0).transpose(1, 0, 2)
        ).astype(NPBF)

    in_maps = []
    for c in range(NCORES):
        f0, f1 = c * HH, (c + 1) * HH
        m = {
            "xt": xt,
            "wq": wsplit(Wq, f0, f1),
            "wk": wsplit(Wk, f0, f1),
            "wv": wsplit(Wv, f0, f1),
            "wo": np.ascontiguousarray(Wo[f0:f1, :]).astype(NPBF),
        }
        if with_bias_qkv:
            m["bqkv"] = np.ascontiguousarray(
                np.stack([bq[f0:f1], bk[f0:f1], bv[f0:f1]])
            )
        if with_bias_o:
            m["bo8"] = np.ascontiguousarray(bo / NCORES)
        in_maps.append(m)

    kwargs = {}
    if _trace:
        kwargs["trace"] = True
        if _trace_cores is not None:
            kwargs["trace_cores"] = _trace_cores
    res = run_bass_kernel_spmd(nc, in_maps, core_ids=list(range(NCORES)), **kwargs)

    acc = np.zeros((B, S, D), dtype=np.float64)
    for c in range(NCORES):
        acc += np.asarray(res.results[c]["out"], dtype=np.float64)
    if not with_bias_o:
        acc += bo  # bo is zero here, but keep the math explicit
    out = acc.astype(np.float32)
    if _trace:
        return out, res
    return out
